# revision 28
# baseline (speedup 1.0000x reference)
"""Trainium2 Bass kernel for LongNet-style dilated attention.

Module config (hardcoded): x [4, 8192, 2048] f32, d_model=2048, 16 heads,
head_dim=128, segment=512, dilation=2.

Math per (batch, segment, head):
  g = x[b, seg, offset_h::2, h*128:(h+1)*128]          # [256, 128]
  A = softmax(g @ g.T / sqrt(128))                      # [256, 256]
  out[b, seg, offset_h::2, h*128:(h+1)*128] = A @ g     # rest stays 0

Sharding: 64 segments (4 batches x 16 segs) split 8-per-core across the
8 NeuronCores; segments are fully independent (no collectives).

Kernel structure per core (8 segments x 2 parities = 16 "groups" of 8
heads; a flattened software pipeline with a 2-head skew keeps every
engine's in-order queue from head-of-line blocking):
  - per group: one 2MB token-major DMA load (parity-strided rows), a
    bf16 shadow copy with a trailing all-ones region, so the A@g matmul
    rhs [g_h | ones] also emits the softmax denominator into PSUM.
  - per head: PE transposes -> gT; S = gT.T@gT in float32r (TF32-like,
    1 cyc/row); one batched exp on ScalarE; 4 bf16 out-matmuls; DVE
    reciprocal of the fused rowsum; normalization folded into the
    PSUM->SBUF output copy on ScalarE.
  - only dilated positions are written back (strided DMA); the
    harness's output buffers are zero-initialized, giving the zeros
    elsewhere.
"""

import numpy as np

import concourse.bacc as bacc
import concourse.bass as bass
import concourse.tile as tile
from concourse import mybir
from concourse.bass_utils import run_bass_kernel_spmd
from concourse.masks import make_identity

N_CORES = 8
B = 4
N_TOK = 8192
D = 2048
H = 16
HD = 128
SEG = 512
SDIL = 256  # dilated tokens per segment per head (SEG / dilation)
SCALE = 1.0 / float(np.sqrt(HD))

SEGS_TOTAL = (B * N_TOK) // SEG  # 64
SEGS_PER_CORE = SEGS_TOTAL // N_CORES  # 8

FP32 = mybir.dt.float32
FP32R = mybir.dt.float32r
BF16 = mybir.dt.bfloat16
EXP = mybir.ActivationFunctionType.Exp

XBW = D + HD  # bf16 shadow width: 2048 g columns + 128-wide ones region


def build_nc(n_segs=SEGS_PER_CORE, s_dtype=BF16, o_dtype=BF16):
    """Build the per-core Bass program for n_segs segments."""
    nc = bacc.Bacc(
        "TRN2", target_bir_lowering=False, debug=False, num_devices=N_CORES
    )
    ntok = n_segs * SEG
    x = nc.dram_tensor("x", [ntok, D], FP32, kind="ExternalInput").ap()
    out = nc.dram_tensor("out", [ntok, D], FP32, kind="ExternalOutput").ap()

    # row n = s*512 + t*2 + u  (u = parity, t = dilated index)
    xv = x.rearrange("(s t u) d -> s u t d", u=2, t=SDIL)
    # col d = hh*256 + uu*128 + c  (head h = 2*hh + uu)
    ov = out.rearrange(
        "(s t u) (hh uu c) -> s u t hh uu c", t=SDIL, u=2, uu=2, c=HD
    )

    n_groups = n_segs * 2
    n_items = n_groups * 8

    with tile.TileContext(nc) as tc:
        with (
            tc.tile_pool(name="xp", bufs=4) as xp_pool,
            tc.tile_pool(name="xb", bufs=3) as xb_pool,
            tc.tile_pool(name="gt", bufs=4) as gt_pool,
            tc.tile_pool(name="ee", bufs=4) as e_pool,
            tc.tile_pool(name="small", bufs=4) as small_pool,
            tc.tile_pool(name="stage", bufs=3) as stage_pool,
            tc.tile_pool(name="const", bufs=1) as const_pool,
            tc.tile_pool(name="gtps", bufs=2, space="PSUM") as gtps_pool,
            tc.tile_pool(name="sps", bufs=2, space="PSUM") as sps_pool,
            tc.tile_pool(name="ops", bufs=4, space="PSUM") as ops_pool,
        ):
            ident = const_pool.tile([128, 128], BF16)
            make_identity(nc, ident)

            G = {}  # group id -> dict of tiles

            def emit_load(g):
                if g >= n_groups:
                    return
                s, u = divmod(g, 2)
                # bf16 x tile with a trailing all-ones region; the fp32->bf16
                # cast happens inside the (SWDGE) DMA engines for free
                xb = xb_pool.tile([128, 2, XBW], BF16, tag="xb")
                nc.gpsimd.dma_start(
                    out=xb[:, :, 0:D],
                    in_=xv[s, u].rearrange("(i t) d -> t i d", i=2),
                )
                nc.vector.memset(xb[:, :, D:XBW], 1.0)
                stage = stage_pool.tile([128, 2, 8, HD], FP32, tag="st")
                G[g] = {"xb": xb, "stage": stage, "s": s, "u": u}

            def emit_cast(g):
                pass

            def rhs_ap(xb, i, h):
                # [g_h (128 cols) | ones...]: 2-level free AP whose second
                # step lands in the all-ones region for every inner index
                base = xb[:, i, h * HD:(h + 1) * HD]
                return bass.AP(
                    tensor=base.tensor,
                    offset=base.offset,
                    ap=[base.ap[0], [D - h * HD, 2], [1, HD]],
                )

            def stage_T(i):
                if i >= n_items:
                    return
                g, hh = divmod(i, 8)
                gd = G[g]
                h = 2 * hh + gd["u"]
                cs = slice(h * HD, (h + 1) * HD)
                gt_ps = gtps_pool.tile([128, 256], BF16)
                nc.tensor.transpose(gt_ps[:, 0:128], gd["xb"][:, 0, cs], ident)
                nc.tensor.transpose(gt_ps[:, 128:256], gd["xb"][:, 1, cs], ident)
                gt = gt_pool.tile([128, 256], s_dtype, tag="gt")
                if hh % 2 == 0:
                    nc.scalar.copy(gt, gt_ps)
                else:
                    nc.vector.tensor_copy(gt, gt_ps)
                gd[("gt", hh)] = gt

            def stage_S(i):
                if i < 0 or i >= n_items:
                    return
                g, hh = divmod(i, 8)
                gd = G[g]
                gt = gd.pop(("gt", hh))
                s_ps = sps_pool.tile([128, 512], FP32)
                nc.tensor.matmul(
                    s_ps[:, 0:256], gt[:, 0:128], gt, start=True, stop=True
                )
                nc.tensor.matmul(
                    s_ps[:, 256:512], gt[:, 128:256], gt, start=True, stop=True
                )
                e = e_pool.tile([128, 512], o_dtype, tag="ee")
                nc.scalar.activation(e, s_ps, EXP, scale=SCALE)
                gd[("e", hh)] = e

            def stage_O(i):
                if i < 0:
                    return
                g, hh = divmod(i, 8)
                gd = G[g]
                h = 2 * hh + gd["u"]
                xb = gd["xb"]
                e = gd.pop(("e", hh))
                o_ps = ops_pool.tile([128, 2, 256], FP32)
                nc.tensor.matmul(
                    o_ps[:, 0, :], e[:, 0:128], rhs_ap(xb, 0, h),
                    start=True, stop=False,
                )
                nc.tensor.matmul(
                    o_ps[:, 0, :], e[:, 256:384], rhs_ap(xb, 1, h),
                    start=False, stop=True,
                )
                nc.tensor.matmul(
                    o_ps[:, 1, :], e[:, 128:256], rhs_ap(xb, 0, h),
                    start=True, stop=False,
                )
                nc.tensor.matmul(
                    o_ps[:, 1, :], e[:, 384:512], rhs_ap(xb, 1, h),
                    start=False, stop=True,
                )
                rcp = small_pool.tile([128, 2], FP32, tag="rcp")
                nc.vector.reciprocal(rcp, o_ps[:, :, HD])
                stage = gd["stage"]
                for qc in range(2):
                    nc.vector.tensor_scalar_mul(
                        stage[:, qc, hh, :], o_ps[:, qc, 0:HD], rcp[:, qc:qc + 1]
                    )
                if hh == 7:
                    s, u = gd["s"], gd["u"]
                    for qc in range(2):
                        eng = nc.sync if qc == 0 else nc.gpsimd
                        eng.dma_start(
                            out=ov[s, u, qc * 128:(qc + 1) * 128, :, u, :],
                            in_=stage[:, qc],
                        )

            # prologue: loads lead by 2 groups, casts by 1
            emit_load(0)
            emit_load(1)
            emit_cast(0)
            for i in range(n_items + 3):
                if i < n_items and i % 8 == 0:
                    g = i // 8
                    emit_load(g + 2)
                    emit_cast(g + 1)
                stage_T(i)
                stage_S(i - 1)
                stage_O(i - 3)

    nc.compile()
    return nc


_NC_CACHE = {}


def _get_nc():
    key = "full"
    if key not in _NC_CACHE:
        _NC_CACHE[key] = build_nc()
    return _NC_CACHE[key]


def make_in_maps(x: np.ndarray):
    xs = np.ascontiguousarray(x).reshape(SEGS_TOTAL, SEG, D)
    in_maps = []
    for c in range(N_CORES):
        chunk = xs[c * SEGS_PER_CORE:(c + 1) * SEGS_PER_CORE]
        in_maps.append(
            {"x": np.ascontiguousarray(chunk).reshape(SEGS_PER_CORE * SEG, D)}
        )
    return in_maps


def gather_out(results) -> np.ndarray:
    outs = [results[c]["out"] for c in range(N_CORES)]
    return np.concatenate(outs, axis=0).reshape(B, N_TOK, D)


def kernel(x: np.ndarray) -> np.ndarray:
    assert x.shape == (B, N_TOK, D) and x.dtype == np.float32
    nc = _get_nc()
    res = run_bass_kernel_spmd(nc, make_in_maps(x), list(range(N_CORES)))
    return gather_out(res.results)


# revision 30
# speedup vs baseline: 1.1028x; 1.1028x over previous
"""Trainium2 Bass kernel for LongNet-style dilated attention.

Module config (hardcoded): x [4, 8192, 2048] f32, d_model=2048, 16 heads,
head_dim=128, segment=512, dilation=2.

Math per (batch, segment, head):
  g = x[b, seg, offset_h::2, h*128:(h+1)*128]          # [256, 128]
  A = softmax(g @ g.T / sqrt(128))                      # [256, 256]
  out[b, seg, offset_h::2, h*128:(h+1)*128] = A @ g     # rest stays 0

Sharding: 64 segments (4 batches x 16 segs) split 8-per-core across the
8 NeuronCores; segments are fully independent (no collectives).

Kernel structure per core (8 segments x 2 parities = 16 "groups" of 8
heads; a flattened software pipeline with a 2-head skew keeps every
engine's in-order queue from head-of-line blocking):
  - per group: one 2MB token-major DMA load (parity-strided rows), a
    bf16 shadow copy with a trailing all-ones region, so the A@g matmul
    rhs [g_h | ones] also emits the softmax denominator into PSUM.
  - per head: PE transposes -> gT; S = gT.T@gT in float32r (TF32-like,
    1 cyc/row); one batched exp on ScalarE; 4 bf16 out-matmuls; DVE
    reciprocal of the fused rowsum; normalization folded into the
    PSUM->SBUF output copy on ScalarE.
  - only dilated positions are written back (strided DMA); the
    harness's output buffers are zero-initialized, giving the zeros
    elsewhere.
"""

import numpy as np

import concourse.bacc as bacc
import concourse.bass as bass
import concourse.tile as tile
from concourse import mybir
from concourse.bass_utils import run_bass_kernel_spmd
from concourse.masks import make_identity

N_CORES = 8
B = 4
N_TOK = 8192
D = 2048
H = 16
HD = 128
SEG = 512
SDIL = 256  # dilated tokens per segment per head (SEG / dilation)
SCALE = 1.0 / float(np.sqrt(HD))

SEGS_TOTAL = (B * N_TOK) // SEG  # 64
SEGS_PER_CORE = SEGS_TOTAL // N_CORES  # 8

FP32 = mybir.dt.float32
FP32R = mybir.dt.float32r
BF16 = mybir.dt.bfloat16
EXP = mybir.ActivationFunctionType.Exp

XBW = D + HD  # bf16 shadow width: 2048 g columns + 128-wide ones region


def build_nc(n_segs=SEGS_PER_CORE, s_dtype=BF16, o_dtype=BF16):
    """Build the per-core Bass program for n_segs segments."""
    nc = bacc.Bacc(
        "TRN2", target_bir_lowering=False, debug=False, num_devices=N_CORES
    )
    ntok = n_segs * SEG
    x = nc.dram_tensor("x", [ntok, D], FP32, kind="ExternalInput").ap()
    out = nc.dram_tensor("out", [ntok, D], FP32, kind="ExternalOutput").ap()

    # row n = s*512 + t*2 + u  (u = parity, t = dilated index)
    xv = x.rearrange("(s t u) d -> s u t d", u=2, t=SDIL)
    # col d = hh*256 + uu*128 + c  (head h = 2*hh + uu)
    ov = out.rearrange(
        "(s t u) (hh uu c) -> s u t hh uu c", t=SDIL, u=2, uu=2, c=HD
    )

    n_groups = n_segs * 2
    n_items = n_groups * 8

    with tile.TileContext(nc) as tc:
        with (
            tc.tile_pool(name="xp", bufs=4) as xp_pool,
            tc.tile_pool(name="xb", bufs=3) as xb_pool,
            tc.tile_pool(name="gt", bufs=4) as gt_pool,
            tc.tile_pool(name="ee", bufs=4) as e_pool,
            tc.tile_pool(name="small", bufs=4) as small_pool,
            tc.tile_pool(name="stage", bufs=3) as stage_pool,
            tc.tile_pool(name="const", bufs=1) as const_pool,
            tc.tile_pool(name="gtps", bufs=2, space="PSUM") as gtps_pool,
            tc.tile_pool(name="sps", bufs=3, space="PSUM") as sps_pool,
            tc.tile_pool(name="ops", bufs=3, space="PSUM") as ops_pool,
        ):
            ident = const_pool.tile([128, 128], BF16)
            make_identity(nc, ident)

            G = {}  # group id -> dict of tiles

            def emit_load(g):
                if g >= n_groups:
                    return
                s, u = divmod(g, 2)
                # bf16 x tile with a trailing all-ones region; the fp32->bf16
                # cast happens inside the (SWDGE) DMA engines for free
                xb = xb_pool.tile([128, 2, XBW], BF16, tag="xb")
                nc.gpsimd.dma_start(
                    out=xb[:, :, 0:D],
                    in_=xv[s, u].rearrange("(i t) d -> t i d", i=2),
                )
                nc.vector.memset(xb[:, :, D:XBW], 1.0)
                stage = stage_pool.tile([128, 2, 8, HD], FP32, tag="st")
                G[g] = {"xb": xb, "stage": stage, "s": s, "u": u}

            def emit_cast(g):
                pass

            def rhs_ap(xb, i, h):
                # [g_h (128 cols) | ones...]: 2-level free AP whose second
                # step lands in the all-ones region for every inner index
                base = xb[:, i, h * HD:(h + 1) * HD]
                return bass.AP(
                    tensor=base.tensor,
                    offset=base.offset,
                    ap=[base.ap[0], [D - h * HD, 2], [1, HD]],
                )

            def stage_T(i):
                if i >= n_items:
                    return
                g, hh = divmod(i, 8)
                gd = G[g]
                h = 2 * hh + gd["u"]
                cs = slice(h * HD, (h + 1) * HD)
                gt_ps = gtps_pool.tile([128, 256], BF16)
                nc.tensor.transpose(gt_ps[:, 0:128], gd["xb"][:, 0, cs], ident)
                nc.tensor.transpose(gt_ps[:, 128:256], gd["xb"][:, 1, cs], ident)
                gt = gt_pool.tile([128, 256], s_dtype, tag="gt")
                if hh % 2 == 0:
                    nc.scalar.copy(gt, gt_ps)
                else:
                    nc.vector.tensor_copy(gt, gt_ps)
                gd[("gt", hh)] = gt

            def stage_S(i):
                if i < 0 or i >= n_items:
                    return
                g, hh = divmod(i, 8)
                gd = G[g]
                gt = gd.pop(("gt", hh))
                s_ps = sps_pool.tile([128, 512], FP32)
                nc.tensor.matmul(
                    s_ps[:, 0:256], gt[:, 0:128], gt, start=True, stop=True
                )
                nc.tensor.matmul(
                    s_ps[:, 256:512], gt[:, 128:256], gt, start=True, stop=True
                )
                e = e_pool.tile([128, 512], o_dtype, tag="ee")
                nc.scalar.activation(e, s_ps, EXP, scale=SCALE)
                gd[("e", hh)] = e

            def stage_O(i):
                if i < 0:
                    return
                g, hh = divmod(i, 8)
                gd = G[g]
                h = 2 * hh + gd["u"]
                xb = gd["xb"]
                e = gd.pop(("e", hh))
                o_ps = ops_pool.tile([128, 2, 256], FP32)
                nc.tensor.matmul(
                    o_ps[:, 0, :], e[:, 0:128], rhs_ap(xb, 0, h),
                    start=True, stop=False,
                )
                nc.tensor.matmul(
                    o_ps[:, 0, :], e[:, 256:384], rhs_ap(xb, 1, h),
                    start=False, stop=True,
                )
                nc.tensor.matmul(
                    o_ps[:, 1, :], e[:, 128:256], rhs_ap(xb, 0, h),
                    start=True, stop=False,
                )
                nc.tensor.matmul(
                    o_ps[:, 1, :], e[:, 384:512], rhs_ap(xb, 1, h),
                    start=False, stop=True,
                )
                rcp = small_pool.tile([128, 2], FP32, tag="rcp")
                nc.vector.reciprocal(rcp, o_ps[:, :, HD])
                stage = gd["stage"]
                for qc in range(2):
                    nc.vector.tensor_scalar_mul(
                        stage[:, qc, hh, :], o_ps[:, qc, 0:HD], rcp[:, qc:qc + 1]
                    )
                if hh == 7:
                    s, u = gd["s"], gd["u"]
                    for qc in range(2):
                        eng = nc.sync if qc == 0 else nc.gpsimd
                        eng.dma_start(
                            out=ov[s, u, qc * 128:(qc + 1) * 128, :, u, :],
                            in_=stage[:, qc],
                        )

            # prologue: loads lead by 2 groups, casts by 1
            emit_load(0)
            emit_load(1)
            emit_cast(0)
            for i in range(n_items + 2):
                if i < n_items and i % 8 == 0:
                    g = i // 8
                    emit_load(g + 2)
                    emit_cast(g + 1)
                stage_T(i)
                stage_S(i - 1)
                stage_O(i - 2)

    nc.compile()
    return nc


_NC_CACHE = {}


def _get_nc():
    key = "full"
    if key not in _NC_CACHE:
        _NC_CACHE[key] = build_nc()
    return _NC_CACHE[key]


def make_in_maps(x: np.ndarray):
    xs = np.ascontiguousarray(x).reshape(SEGS_TOTAL, SEG, D)
    in_maps = []
    for c in range(N_CORES):
        chunk = xs[c * SEGS_PER_CORE:(c + 1) * SEGS_PER_CORE]
        in_maps.append(
            {"x": np.ascontiguousarray(chunk).reshape(SEGS_PER_CORE * SEG, D)}
        )
    return in_maps


def gather_out(results) -> np.ndarray:
    outs = [results[c]["out"] for c in range(N_CORES)]
    return np.concatenate(outs, axis=0).reshape(B, N_TOK, D)


def kernel(x: np.ndarray) -> np.ndarray:
    assert x.shape == (B, N_TOK, D) and x.dtype == np.float32
    nc = _get_nc()
    res = run_bass_kernel_spmd(nc, make_in_maps(x), list(range(N_CORES)))
    return gather_out(res.results)
